# revision 20
# baseline (speedup 1.0000x reference)
"""AllAtomFAPE loss kernel for Trainium2 (8 NeuronCores, SPMD) — v2.

Problem: b=1, N=384 res, F=8 frames/res -> NF=3072 frames; A=14 atoms/res
-> NA=5376 atoms. Output: scalar masked clamped FAPE.

Algorithm (factorized pairwise distance):
  With P = pR pR^T, T = tR tR^T, M = pR tR^T (per frame, 3x3),
    d2(f,a) = (pp-pt)^T P (pp-pt) + (tp-tt)^T T (tp-tt)
              - 2 (pp-pt)^T M (tp-tt)
  expands into a K=39 dot product (rows padded to even offsets so the
  bf16 atom-feature writes stay 4B-aligned; pad rows are zero on both
  sides) between frame features W[:,f] and atom features Z[:,a]:
    rows 0-8   : P[i,j]          <->  pp_i pp_j
    rows 10-18 : M[i,j]          <->  -2 pp_i tp_j
    rows 20-28 : T[i,j]          <->  tp_i tp_j
    rows 30-32 : (M tt - P pt)   <->  2 pp
    rows 34-36 : (M^T pt - T tt) <->  2 tp
    row  38    : c_f + EPS       <->  m_a^2   (mask + eps folded in, so
                                      PSUM d2 = m^2 (d2_true + eps))
  ScalarE computes d = sqrt(psum) with no scale/bias, DVE clamps with a
  constant min(d, 10) (exact for binary masks: m=0 -> d=0), and sums go
  through PE ones-matmuls (most tiles) or the fused DVE min+accum.

Sharding: FRAMES sharded across the 8 cores (384 frames = 3 partition
blocks each, no padding); atoms replicated (5376 = 42 blocks as the
moving free dim). Each core emits [S_partial, mask_sum]; the host does
S_total * CNORM / (eps + mask_sum).

Layouts: PE-transposed via identity matmuls.  W slab holds each frame
block duplicated into both 64-row slots of its group (col = 128*g +
64*s + r) so a stationary exists at base partition 0 AND 64; the
stationary for (fb, u) is WTi[64*u + (0:39), 128*fb + (0:128)].
Atom slab uses pairing (g, g+21): col = 128*g + 64*u + r with block
t = g + 21*u, so ZTi[64*u + (0:39), :] is a contiguous [39, 2688]
moving operand for atom half u.
"""

import numpy as np

import concourse.bacc as bacc
import concourse.bass as bass
import concourse.tile as tile
from concourse import mybir
from concourse.bass_utils import run_bass_kernel_spmd

F32 = mybir.dt.float32
BF16 = mybir.dt.bfloat16
AX = mybir.AxisListType
OP = mybir.AluOpType
ACTF = mybir.ActivationFunctionType

NCORES = 8
NF = 3072
NFC = NF // NCORES     # 384 frames per core
NFB = NFC // 128       # 3 frame blocks
NA = 5376
NAB = NA // 128        # 42 atom blocks
GZ = NAB // 2          # 21 atom groups
K = 39                 # contract rows (with pads)
KS = 64
CHA = 1344             # atom cols per main tile (4 tiles per frame block)
NCH = NA // CHA        # 4
NT = NFB * NCH         # 12 main tiles
EPS = 1e-4
CLAMP = 10.0
ZSCALE = 10.0
CNORM = float(1.0 / (ZSCALE * (3072.0 + EPS)))

# per-tile clamp+sum: tensor_tensor_reduce min-vs-10s-tile with fused
# accumulate (TT form can run the 2x bf16 DVE mode; tensor_scalar+accum
# is stuck at 1x).
GP_TILES = ()


def _bc(ap, dim, n):
    """Broadcast AP along a new axis at position `dim` (stride-0), n copies."""
    return ap.unsqueeze(dim).to_broadcast(
        tuple(ap.shape[:dim]) + (n,) + tuple(ap.shape[dim:])
    )


def build_nc():
    nc = bacc.Bacc(None)

    FRW = 9 * NFB * 2 + 3 * NFB * 2          # 72 cols fp32
    ATW = 3 * NAB * 2 + NAB + 128            # 422 cols bf16 (incl identity)
    fr_d = nc.declare_dram_parameter("fr", [128, FRW], F32, isOutput=False)
    at_d = nc.declare_dram_parameter("at", [128, ATW], BF16, isOutput=False)
    out_d = nc.declare_dram_parameter("out", [2, 1], F32, isOutput=True)

    with tile.TileContext(nc) as tc:
        with (
            tc.tile_pool(name="consts", bufs=1) as consts,
            tc.tile_pool(name="sbuf_s", bufs=3) as sbuf_s,
        ):
            # ---------------- input DMAs --------------------------------
            ats = consts.tile([128, ATW], BF16)
            frs = consts.tile([128, FRW], F32)
            nc.sync.dma_start(out=ats[:, 0:252], in_=at_d[:, 0:252])
            nc.gpsimd.dma_start(out=frs[:], in_=fr_d[:])
            nc.scalar.dma_start(out=ats[:, 252:422], in_=at_d[:, 252:422])
            # touch Sqrt immediately so its ACT table loads during the
            # DMA wait instead of right before the first main-loop sqrt
            tbl = consts.tile([1, 1], F32)
            nc.vector.memset(tbl[:], 0.0)
            nc.scalar.activation(out=tbl[:], in_=tbl[:], func=ACTF.Sqrt)
            at_ap = ats[:]
            ppb = at_ap[:, 0:126]                  # col = c*42 + t
            tpb = at_ap[:, 126:252]
            amb = at_ap[:, 252:294]
            ident = at_ap[:, 294:422]
            fr_ap = frs[:]
            ptb = fr_ap[:, 54:63]                  # col = 54 + 3c + t
            ttb = fr_ap[:, 63:72]

            # ---------------- atom (Z) features, bf16 -------------------
            Zslab = consts.tile([128, 128 * GZ], BF16)
            nc.vector.memset(Zslab[:], 0.0)
            n2pp = consts.tile([128, 126], BF16)
            nc.vector.tensor_scalar_mul(n2pp[:], ppb, -2.0)

            pp3 = ppb.rearrange("p (c t) -> p c t", c=3)       # [128,3,42]
            tp3 = tpb.rearrange("p (c t) -> p c t", c=3)
            n2pp3 = n2pp[:].rearrange("p (c t) -> p c t", c=3)
            Zg = Zslab[:].rearrange("p (g v) -> p g v", v=128)  # [128,21,128]

            def z_feats(u):
                lo, hi = GZ * u, GZ * u + GZ
                b0 = 64 * u
                # [p, g, c] operand views (transpose puts g before c)
                ppu = pp3[:, :, lo:hi].transpose([0, 2, 1])
                tpu = tp3[:, :, lo:hi].transpose([0, 2, 1])
                n2u = n2pp3[:, :, lo:hi].transpose([0, 2, 1])

                def quad(r0, av, bv):
                    out = Zg[:, :, b0 + r0:b0 + r0 + 9].rearrange(
                        "p g (i j) -> p g i j", i=3)
                    nc.vector.tensor_mul(out, _bc(av, 3, 3), _bc(bv, 2, 3))

                quad(0, ppu, ppu)     # pp_i pp_j
                quad(10, n2u, tpu)    # -2 pp_i tp_j
                quad(20, tpu, tpu)    # tp_i tp_j
                nc.vector.tensor_scalar_mul(
                    Zg[:, :, b0 + 30:b0 + 33], ppu, 2.0)
                nc.vector.tensor_scalar_mul(
                    Zg[:, :, b0 + 34:b0 + 37], tpu, 2.0)
                amu = amb[:, lo:hi].unsqueeze(2)
                nc.vector.tensor_mul(Zg[:, :, b0 + 38:b0 + 39], amu, amu)

            # ---------------- frame (W) features, fp32 ------------------
            Wslab = consts.tile([128, KS * NFB], F32)   # col = 64*t + r
            # bf16 slab duplicates each block into both 64-row slots of
            # its group: col = 128*g + 64*s + r, block fb = g, s in {0,1}
            Wslab_b = consts.tile([128, 128 * NFB], BF16)
            nc.vector.memset(Wslab[:], 0.0)
            nc.vector.memset(Wslab_b[:], 0.0)

            W4 = Wslab[:].rearrange("p (t r) -> p r t", r=KS)
            R9 = fr_ap[:, 0:27].rearrange("p (c t) -> p c t", c=9)
            T9 = fr_ap[:, 27:54].rearrange("p (c t) -> p c t", c=9)
            pt3 = ptb.rearrange("p (c t) -> p c t", c=3)
            tt3 = ttb.rearrange("p (c t) -> p c t", c=3)

            m54 = consts.tile([128, 54], F32)
            m54b = consts.tile([128, 54], F32)
            m54c = consts.tile([128, 54], F32)
            mA = consts.tile([128, 18], F32)     # [i, q, t]: P.pt | M^T.pt
            mB = consts.tile([128, 18], F32)     # [i, q, t]: M.tt | T.tt
            t18 = consts.tile([128, 18], F32)
            t9 = consts.tile([128, 9], F32)
            t9b = consts.tile([128, 9], F32)
            t3 = consts.tile([128, 3], F32)

            def w_feats():
                # grams: P (rows 0:9) = R R^T, M (10:19) = R T^T,
                # T (20:29) = T T^T   (ops capped at 3 free dims)
                ma3 = m54[:, 0:27].rearrange("p (i j t) -> p i j t", i=3, j=3)
                mb3 = m54b[:, 0:27].rearrange("p (i j t) -> p i j t", i=3, j=3)
                mc3 = m54c[:, 0:27].rearrange("p (i j t) -> p i j t", i=3, j=3)
                Ri = R9.rearrange("p (i k) t -> p i k t", i=3)
                Ti = T9.rearrange("p (j k) t -> p j k t", j=3)

                def gram(rows, A4, B4):
                    out = W4[:, rows:rows + 9, :].rearrange(
                        "p (i j) t -> p i j t", i=3)
                    a = lambda k: _bc(A4[:, :, k, :], 2, 3)
                    b = lambda k: _bc(B4[:, :, k, :], 1, 3)
                    nc.vector.tensor_mul(ma3, a(0), b(0))
                    nc.vector.tensor_mul(mb3, a(1), b(1))
                    nc.vector.tensor_mul(mc3, a(2), b(2))
                    nc.vector.tensor_add(ma3, ma3, mb3)
                    nc.vector.tensor_add(out, ma3, mc3)

                gram(0, Ri, Ri)
                gram(10, Ri, Ti)
                gram(20, Ti, Ti)

                # matvec pair A: out[i,q,t] = sum_j W[10q+3j+i] * pt_j
                #   q=0: P.pt (P sym, read P_ji); q=1: M^T.pt (read M_ji)
                mv = m54[:].rearrange("p (j i q t) -> p j i q t", j=3, i=3, q=2)
                WqA = W4[:, 0:20, :].rearrange(
                    "p (q h) t -> p q h t", q=2)[:, :, 0:9, :].rearrange(
                    "p q (j i) t -> p j i q t", j=3)
                mAv = mA[:].rearrange("p (i q t) -> p i q t", i=3, q=2)
                ptj = _bc(pt3, 2, 3)                           # [p,j,i,t]
                nc.vector.tensor_mul(mv[:, :, :, 0, :], WqA[:, :, :, 0, :], ptj)
                nc.vector.tensor_mul(mv[:, :, :, 1, :], WqA[:, :, :, 1, :], ptj)
                nc.vector.tensor_add(mAv, mv[:, 0], mv[:, 1])
                nc.vector.tensor_add(mAv, mAv, mv[:, 2])

                # matvec pair B: out[i,q,t] = sum_j W[10+10q+3i+j] * tt_j
                #   q=0: M.tt; q=1: T.tt (T sym)
                WqB = W4[:, 10:30, :].rearrange(
                    "p (q h) t -> p q h t", q=2)[:, :, 0:9, :].rearrange(
                    "p q (i j) t -> p j i q t", i=3)
                mBv = mB[:].rearrange("p (i q t) -> p i q t", i=3, q=2)
                ttj = _bc(tt3, 2, 3)
                nc.vector.tensor_mul(mv[:, :, :, 0, :], WqB[:, :, :, 0, :], ttj)
                nc.vector.tensor_mul(mv[:, :, :, 1, :], WqB[:, :, :, 1, :], ttj)
                nc.vector.tensor_add(mBv, mv[:, 0], mv[:, 1])
                nc.vector.tensor_add(mBv, mBv, mv[:, 2])

                # rows 30:33 = M.tt - P.pt ; rows 34:37 = M^T.pt - T.tt
                nc.vector.tensor_sub(
                    W4[:, 30:33, :], mBv[:, :, 0, :], mAv[:, :, 0, :])
                nc.vector.tensor_sub(
                    W4[:, 34:37, :], mAv[:, :, 1, :], mBv[:, :, 1, :])

                # cf row 38 = pt.(P pt) - 2 pt.(M tt) + tt.(T tt) + EPS
                ptt = fr_ap[:, 54:72].rearrange(
                    "p (q i t) -> p i q t", q=2, i=3)          # pt | tt
                mBm = t18[:].rearrange("p (i q t) -> p i q t", i=3, q=2)
                nc.vector.tensor_mul(mBm, mBv, ptt)  # [pt.Mtt_i | tt.Ttt_i]
                mAm = t9[:].rearrange("p (i t) -> p i t", i=3)
                nc.vector.tensor_mul(mAm, mAv[:, :, 0, :], ptt[:, :, 0, :])
                t9v = t9b[:].rearrange("p (i t) -> p i t", i=3)
                nc.vector.scalar_tensor_tensor(
                    out=t9v, in0=mBm[:, :, 0, :], scalar=-2.0, in1=mAm,
                    op0=OP.mult, op1=OP.add)
                nc.vector.tensor_add(t9v, t9v, mBm[:, :, 1, :])
                nc.vector.tensor_add(t3[:], t9b[:, 0:3], t9b[:, 3:6])
                nc.vector.scalar_tensor_tensor(
                    out=W4[:, 38, :], in0=t3[:], scalar=EPS, in1=t9v[:, 2, :],
                    op0=OP.add, op1=OP.add)

            z_feats(0)
            z_feats(1)
            w_feats()
            Wb4 = Wslab_b[:].rearrange("p (g s r) -> p g s r", s=2, r=KS)
            Wf4 = Wslab[:].rearrange("p (t r) -> p t r", r=KS)

            # ---------------- transposes (PE + ACT copies) --------------
            identity = consts.tile([128, 128], BF16)
            nc.scalar.copy(identity[:], ident)
            WTi = consts.tile([128, 128 * NFB], BF16)
            ZTi = consts.tile([128, 128 * GZ], BF16)
            with tc.tile_pool(name="pst", bufs=2, space="PSUM") as pst_pool:
                def transpose3(dst, src, glo, ghi):
                    pst = pst_pool.tile([128, 384], BF16, tag="tp")
                    for g in range(glo, ghi):
                        nc.tensor.transpose(
                            pst[:, 128 * (g - glo):128 * (g - glo + 1)],
                            src[:, 128 * g:128 * (g + 1)], identity[:])
                    nw = 128 * (ghi - glo)
                    nc.scalar.copy(dst[:, 128 * glo:128 * ghi], pst[:, 0:nw])

                for pz in range(7):
                    transpose3(ZTi[:], Zslab[:], 3 * pz, 3 * pz + 3)
                # W: per-group cast+transpose+copy so the fb0 stationary is
                # ready as early as possible (cast rows 0:39 dup'd into both
                # slots; the rest of Wslab_b stays 0)
                for g in range(NFB):
                    nc.vector.tensor_copy(
                        Wb4[:, g:g + 1, :, 0:K],
                        _bc(Wf4[:, g:g + 1, 0:K], 2, 2))
                    pstw = pst_pool.tile([128, 384], BF16, tag="tp")
                    nc.tensor.transpose(
                        pstw[:, 0:128], Wslab_b[:, 128 * g:128 * (g + 1)],
                        identity[:])
                    nc.scalar.copy(
                        WTi[:, 128 * g:128 * (g + 1)], pstw[:, 0:128])

            # ---------------- main loop ---------------------------------
            with (
                tc.tile_pool(name="psm", bufs=2, space="PSUM") as psm_pool,
                tc.tile_pool(name="pso", bufs=1, space="PSUM") as pso_pool,
            ):
                colacc = consts.tile([128, NT], F32)
                nc.vector.memset(colacc[:], 0.0)
                tens10 = consts.tile([128, CHA], BF16)
                nc.vector.memset(tens10[:], CLAMP)

                idx = 0
                for c in range(NCH):
                    u = c // 2
                    alo = CHA * (c % 2)
                    mv_ap = ZTi[64 * u:64 * u + K, alo:alo + CHA]
                    for fb in range(NFB):
                        st_ap = WTi[64 * u:64 * u + K,
                                    128 * fb:128 * fb + 128]
                        ps = psm_pool.tile([128, CHA], F32, tag="main")
                        for mlo in range(0, CHA, 512):
                            mhi = min(mlo + 512, CHA)
                            nc.tensor.matmul(
                                ps[:, mlo:mhi], st_ap, mv_ap[:, mlo:mhi])
                        s = sbuf_s.tile([128, CHA], BF16)
                        nc.scalar.activation(
                            out=s[:], in_=ps[:], func=ACTF.Sqrt)
                        d = sbuf_s.tile([128, CHA], BF16, tag="dmin")
                        nc.vector.tensor_scalar(
                            out=d[:], in0=s[:], scalar1=CLAMP,
                            scalar2=None, op0=OP.min, op1=OP.add,
                            accum_out=colacc[:, idx:idx + 1])
                        idx += 1

                # ---------------- epilogue ------------------------------
                ScMc = consts.tile([128, 2], F32)
                nc.vector.reduce_sum(
                    out=ScMc[:, 0:1], in_=colacc[:], axis=AX.X)
                nc.vector.reduce_sum(out=ScMc[:, 1:2], in_=amb, axis=AX.X)
                ones_f = consts.tile([128, 1], F32)
                nc.vector.memset(ones_f[:], 1.0)
                psfin = pso_pool.tile([2, 1], F32, tag="fin")
                nc.tensor.matmul(psfin[:], ScMc[:], ones_f[:])
                res = consts.tile([2, 1], F32)
                nc.vector.tensor_copy(res[:], psfin[:])
                nc.sync.dma_start(out=out_d[:], in_=res[:])

    nc.compile()
    return nc


def prep_in_maps(inputs):
    """Full (unsharded) numpy inputs -> per-core input dicts.

    fr: per-core frame slice, [128, 72] f32, col = comp*3 + fb where the
        local frame index is 128*fb + p.
    at: atoms replicated, [128, 422] bf16, col = comp*42 + t (t = a//128,
        p = a%128), then mask [42], then a 128x128 identity.
    """
    import ml_dtypes
    f32 = np.float32
    bf16 = ml_dtypes.bfloat16

    def fr_c(x, comps, c):
        a = np.asarray(x, f32).reshape(NF, comps)[NFC * c:NFC * (c + 1)]
        return np.ascontiguousarray(
            a.reshape(NFB, 128, comps).transpose(1, 2, 0)).reshape(128, -1)

    def at_full(x, comps):
        a = np.asarray(x, f32).reshape(NA, comps)
        return np.ascontiguousarray(
            a.reshape(NAB, 128, comps).transpose(1, 2, 0)).reshape(128, -1)

    pp = at_full(inputs["predicted_atom_positions"], 3)
    tp = at_full(inputs["true_atom_positions"], 3)
    am = np.ascontiguousarray(
        np.asarray(inputs["atom_mask"], f32).reshape(NAB, 128).T)
    at = np.concatenate(
        [pp, tp, am, np.eye(128, dtype=f32)], axis=1).astype(bf16)

    in_maps = []
    for c in range(NCORES):
        fr = np.ascontiguousarray(np.concatenate([
            fr_c(inputs["predicted_frames_R"], 9, c),
            fr_c(inputs["true_frames_R"], 9, c),
            fr_c(inputs["predicted_frames_t"], 3, c),
            fr_c(inputs["true_frames_t"], 3, c),
        ], axis=1))
        in_maps.append({"fr": fr, "at": at})
    return in_maps


_NC_CACHE = None


def _get_nc():
    global _NC_CACHE
    if _NC_CACHE is None:
        _NC_CACHE = build_nc()
    return _NC_CACHE


def kernel(**inputs):
    nc = _get_nc()
    in_maps = prep_in_maps(inputs)
    r = run_bass_kernel_spmd(nc, in_maps, core_ids=list(range(NCORES)))
    S = np.float64(0.0)
    M = np.float64(0.0)
    for i in range(NCORES):
        S += np.float64(r.results[i]["out"][0, 0])
        M = np.float64(r.results[i]["out"][1, 0])
    total = S * CNORM / (EPS + M)
    return np.array([total], dtype=np.float32)


# revision 21
# speedup vs baseline: 1.1799x; 1.1799x over previous
"""AllAtomFAPE loss kernel for Trainium2 (8 NeuronCores, SPMD) — v2.

Problem: b=1, N=384 res, F=8 frames/res -> NF=3072 frames; A=14 atoms/res
-> NA=5376 atoms. Output: scalar masked clamped FAPE.

Algorithm (factorized pairwise distance):
  With P = pR pR^T, T = tR tR^T, M = pR tR^T (per frame, 3x3),
    d2(f,a) = (pp-pt)^T P (pp-pt) + (tp-tt)^T T (tp-tt)
              - 2 (pp-pt)^T M (tp-tt)
  expands into a K=39 dot product (rows padded to even offsets so the
  bf16 atom-feature writes stay 4B-aligned; pad rows are zero on both
  sides) between frame features W[:,f] and atom features Z[:,a]:
    rows 0-8   : P[i,j]          <->  pp_i pp_j
    rows 10-18 : M[i,j]          <->  -2 pp_i tp_j
    rows 20-28 : T[i,j]          <->  tp_i tp_j
    rows 30-32 : (M tt - P pt)   <->  2 pp
    rows 34-36 : (M^T pt - T tt) <->  2 tp
    row  38    : c_f + EPS       <->  m_a^2   (mask + eps folded in, so
                                      PSUM d2 = m^2 (d2_true + eps))
  ScalarE computes d = sqrt(psum) with no scale/bias, DVE clamps with a
  constant min(d, 10) (exact for binary masks: m=0 -> d=0), and sums go
  through PE ones-matmuls (most tiles) or the fused DVE min+accum.

Sharding: FRAMES sharded across the 8 cores (384 frames = 3 partition
blocks each, no padding); atoms replicated (5376 = 42 blocks as the
moving free dim). Each core emits [S_partial, mask_sum]; the host does
S_total * CNORM / (eps + mask_sum).

Layouts: PE-transposed via identity matmuls.  W slab holds each frame
block duplicated into both 64-row slots of its group (col = 128*g +
64*s + r) so a stationary exists at base partition 0 AND 64; the
stationary for (fb, u) is WTi[64*u + (0:39), 128*fb + (0:128)].
Atom slab uses pairing (g, g+21): col = 128*g + 64*u + r with block
t = g + 21*u, so ZTi[64*u + (0:39), :] is a contiguous [39, 2688]
moving operand for atom half u.
"""

import numpy as np

import concourse.bacc as bacc
import concourse.bass as bass
import concourse.tile as tile
from concourse import mybir
from concourse.bass_utils import run_bass_kernel_spmd

F32 = mybir.dt.float32
BF16 = mybir.dt.bfloat16
AX = mybir.AxisListType
OP = mybir.AluOpType
ACTF = mybir.ActivationFunctionType

NCORES = 8
NF = 3072
NFC = NF // NCORES     # 384 frames per core
NFB = NFC // 128       # 3 frame blocks
NA = 5376
NAB = NA // 128        # 42 atom blocks
GZ = NAB // 2          # 21 atom groups
K = 39                 # contract rows (with pads)
KS = 64
CHA = 1344             # atom cols per main tile (4 tiles per frame block)
NCH = NA // CHA        # 4
NT = NFB * NCH         # 12 main tiles
EPS = 1e-4
CLAMP = 10.0
ZSCALE = 10.0
CNORM = float(1.0 / (ZSCALE * (3072.0 + EPS)))

# per-tile clamp+sum: tensor_tensor_reduce min-vs-10s-tile with fused
# accumulate (TT form can run the 2x bf16 DVE mode; tensor_scalar+accum
# is stuck at 1x).
GP_TILES = ()


def _bc(ap, dim, n):
    """Broadcast AP along a new axis at position `dim` (stride-0), n copies."""
    return ap.unsqueeze(dim).to_broadcast(
        tuple(ap.shape[:dim]) + (n,) + tuple(ap.shape[dim:])
    )


def build_nc():
    nc = bacc.Bacc(None)

    FRW = 9 * NFB * 2 + 3 * NFB * 2          # 72 cols fp32
    ATW = 3 * NAB * 2 + NAB + 128            # 422 cols bf16 (incl identity)
    fr_d = nc.declare_dram_parameter("fr", [128, FRW], F32, isOutput=False)
    at_d = nc.declare_dram_parameter("at", [128, ATW], BF16, isOutput=False)
    out_d = nc.declare_dram_parameter("out", [2, 1], F32, isOutput=True)

    with tile.TileContext(nc) as tc:
        with (
            tc.tile_pool(name="consts", bufs=1) as consts,
            tc.tile_pool(name="sbuf_s", bufs=3) as sbuf_s,
        ):
            # ---------------- input DMAs --------------------------------
            ats = consts.tile([128, ATW], BF16)
            frs = consts.tile([128, FRW], F32)
            nc.sync.dma_start(out=ats[:, 0:252], in_=at_d[:, 0:252])
            nc.gpsimd.dma_start(out=frs[:], in_=fr_d[:])
            nc.scalar.dma_start(out=ats[:, 252:422], in_=at_d[:, 252:422])
            # touch Sqrt immediately so its ACT table loads during the
            # DMA wait instead of right before the first main-loop sqrt
            tbl = consts.tile([1, 1], F32)
            nc.vector.memset(tbl[:], 0.0)
            nc.scalar.activation(out=tbl[:], in_=tbl[:], func=ACTF.Sqrt)
            at_ap = ats[:]
            ppb = at_ap[:, 0:126]                  # col = c*42 + t
            tpb = at_ap[:, 126:252]
            amb = at_ap[:, 252:294]
            ident = at_ap[:, 294:422]
            fr_ap = frs[:]
            ptb = fr_ap[:, 54:63]                  # col = 54 + 3c + t
            ttb = fr_ap[:, 63:72]

            # ---------------- atom (Z) features, bf16 -------------------
            Zslab = consts.tile([128, 128 * GZ], BF16)
            nc.vector.memset(Zslab[:], 0.0)
            n2pp = consts.tile([128, 126], BF16)
            nc.vector.tensor_scalar_mul(n2pp[:], ppb, -2.0)

            pp3 = ppb.rearrange("p (c t) -> p c t", c=3)       # [128,3,42]
            tp3 = tpb.rearrange("p (c t) -> p c t", c=3)
            n2pp3 = n2pp[:].rearrange("p (c t) -> p c t", c=3)
            Zg = Zslab[:].rearrange("p (g v) -> p g v", v=128)  # [128,21,128]

            def z_feats(u):
                lo, hi = GZ * u, GZ * u + GZ
                b0 = 64 * u
                # [p, g, c] operand views (transpose puts g before c)
                ppu = pp3[:, :, lo:hi].transpose([0, 2, 1])
                tpu = tp3[:, :, lo:hi].transpose([0, 2, 1])
                n2u = n2pp3[:, :, lo:hi].transpose([0, 2, 1])

                def quad(r0, av, bv):
                    out = Zg[:, :, b0 + r0:b0 + r0 + 9].rearrange(
                        "p g (i j) -> p g i j", i=3)
                    nc.vector.tensor_mul(out, _bc(av, 3, 3), _bc(bv, 2, 3))

                quad(0, ppu, ppu)     # pp_i pp_j
                quad(10, n2u, tpu)    # -2 pp_i tp_j
                quad(20, tpu, tpu)    # tp_i tp_j
                nc.vector.tensor_scalar_mul(
                    Zg[:, :, b0 + 30:b0 + 33], ppu, 2.0)
                nc.vector.tensor_scalar_mul(
                    Zg[:, :, b0 + 34:b0 + 37], tpu, 2.0)
                amu = amb[:, lo:hi].unsqueeze(2)
                nc.vector.tensor_mul(Zg[:, :, b0 + 38:b0 + 39], amu, amu)

            # ---------------- frame (W) features, fp32 ------------------
            Wslab = consts.tile([128, KS * NFB], F32)   # col = 64*t + r
            # bf16 slab duplicates each block into both 64-row slots of
            # its group: col = 128*g + 64*s + r, block fb = g, s in {0,1}
            Wslab_b = consts.tile([128, 128 * NFB], BF16)
            nc.vector.memset(Wslab[:], 0.0)
            nc.vector.memset(Wslab_b[:], 0.0)

            W4 = Wslab[:].rearrange("p (t r) -> p r t", r=KS)
            R9 = fr_ap[:, 0:27].rearrange("p (c t) -> p c t", c=9)
            T9 = fr_ap[:, 27:54].rearrange("p (c t) -> p c t", c=9)
            pt3 = ptb.rearrange("p (c t) -> p c t", c=3)
            tt3 = ttb.rearrange("p (c t) -> p c t", c=3)

            m54 = consts.tile([128, 54], F32)
            m54b = consts.tile([128, 54], F32)
            m54c = consts.tile([128, 54], F32)
            mA = consts.tile([128, 18], F32)     # [i, q, t]: P.pt | M^T.pt
            mB = consts.tile([128, 18], F32)     # [i, q, t]: M.tt | T.tt
            t18 = consts.tile([128, 18], F32)
            t9 = consts.tile([128, 9], F32)
            t9b = consts.tile([128, 9], F32)
            t3 = consts.tile([128, 3], F32)

            def w_feats():
                # grams: P (rows 0:9) = R R^T, M (10:19) = R T^T,
                # T (20:29) = T T^T   (ops capped at 3 free dims)
                ma3 = m54[:, 0:27].rearrange("p (i j t) -> p i j t", i=3, j=3)
                mb3 = m54b[:, 0:27].rearrange("p (i j t) -> p i j t", i=3, j=3)
                mc3 = m54c[:, 0:27].rearrange("p (i j t) -> p i j t", i=3, j=3)
                Ri = R9.rearrange("p (i k) t -> p i k t", i=3)
                Ti = T9.rearrange("p (j k) t -> p j k t", j=3)

                def gram(rows, A4, B4):
                    out = W4[:, rows:rows + 9, :].rearrange(
                        "p (i j) t -> p i j t", i=3)
                    a = lambda k: _bc(A4[:, :, k, :], 2, 3)
                    b = lambda k: _bc(B4[:, :, k, :], 1, 3)
                    nc.vector.tensor_mul(ma3, a(0), b(0))
                    nc.vector.tensor_mul(mb3, a(1), b(1))
                    nc.vector.tensor_mul(mc3, a(2), b(2))
                    nc.vector.tensor_add(ma3, ma3, mb3)
                    nc.vector.tensor_add(out, ma3, mc3)

                gram(0, Ri, Ri)
                gram(10, Ri, Ti)
                gram(20, Ti, Ti)

                # matvec pair A: out[i,q,t] = sum_j W[10q+3j+i] * pt_j
                #   q=0: P.pt (P sym, read P_ji); q=1: M^T.pt (read M_ji)
                mv = m54[:].rearrange("p (j i q t) -> p j i q t", j=3, i=3, q=2)
                WqA = W4[:, 0:20, :].rearrange(
                    "p (q h) t -> p q h t", q=2)[:, :, 0:9, :].rearrange(
                    "p q (j i) t -> p j i q t", j=3)
                mAv = mA[:].rearrange("p (i q t) -> p i q t", i=3, q=2)
                ptj = _bc(pt3, 2, 3)                           # [p,j,i,t]
                nc.vector.tensor_mul(mv[:, :, :, 0, :], WqA[:, :, :, 0, :], ptj)
                nc.vector.tensor_mul(mv[:, :, :, 1, :], WqA[:, :, :, 1, :], ptj)
                nc.vector.tensor_add(mAv, mv[:, 0], mv[:, 1])
                nc.vector.tensor_add(mAv, mAv, mv[:, 2])

                # matvec pair B: out[i,q,t] = sum_j W[10+10q+3i+j] * tt_j
                #   q=0: M.tt; q=1: T.tt (T sym)
                WqB = W4[:, 10:30, :].rearrange(
                    "p (q h) t -> p q h t", q=2)[:, :, 0:9, :].rearrange(
                    "p q (i j) t -> p j i q t", i=3)
                mBv = mB[:].rearrange("p (i q t) -> p i q t", i=3, q=2)
                ttj = _bc(tt3, 2, 3)
                nc.vector.tensor_mul(mv[:, :, :, 0, :], WqB[:, :, :, 0, :], ttj)
                nc.vector.tensor_mul(mv[:, :, :, 1, :], WqB[:, :, :, 1, :], ttj)
                nc.vector.tensor_add(mBv, mv[:, 0], mv[:, 1])
                nc.vector.tensor_add(mBv, mBv, mv[:, 2])

                # rows 30:33 = M.tt - P.pt ; rows 34:37 = M^T.pt - T.tt
                nc.vector.tensor_sub(
                    W4[:, 30:33, :], mBv[:, :, 0, :], mAv[:, :, 0, :])
                nc.vector.tensor_sub(
                    W4[:, 34:37, :], mAv[:, :, 1, :], mBv[:, :, 1, :])

                # cf row 38 = pt.(P pt) - 2 pt.(M tt) + tt.(T tt) + EPS
                ptt = fr_ap[:, 54:72].rearrange(
                    "p (q i t) -> p i q t", q=2, i=3)          # pt | tt
                mBm = t18[:].rearrange("p (i q t) -> p i q t", i=3, q=2)
                nc.vector.tensor_mul(mBm, mBv, ptt)  # [pt.Mtt_i | tt.Ttt_i]
                mAm = t9[:].rearrange("p (i t) -> p i t", i=3)
                nc.vector.tensor_mul(mAm, mAv[:, :, 0, :], ptt[:, :, 0, :])
                t9v = t9b[:].rearrange("p (i t) -> p i t", i=3)
                nc.vector.scalar_tensor_tensor(
                    out=t9v, in0=mBm[:, :, 0, :], scalar=-2.0, in1=mAm,
                    op0=OP.mult, op1=OP.add)
                nc.vector.tensor_add(t9v, t9v, mBm[:, :, 1, :])
                nc.vector.tensor_add(t3[:], t9b[:, 0:3], t9b[:, 3:6])
                nc.vector.scalar_tensor_tensor(
                    out=W4[:, 38, :], in0=t3[:], scalar=EPS, in1=t9v[:, 2, :],
                    op0=OP.add, op1=OP.add)

            z_feats(0)
            z_feats(1)
            w_feats()
            Wb4 = Wslab_b[:].rearrange("p (g s r) -> p g s r", s=2, r=KS)
            Wf4 = Wslab[:].rearrange("p (t r) -> p t r", r=KS)

            # ---------------- transposes (PE + ACT copies) --------------
            identity = consts.tile([128, 128], BF16)
            nc.scalar.copy(identity[:], ident)
            WTi = consts.tile([128, 128 * NFB], BF16)
            ZTi = consts.tile([128, 128 * GZ], BF16)
            with (
                tc.tile_pool(name="pst", bufs=2, space="PSUM") as pst_pool,
                tc.tile_pool(name="psm", bufs=2, space="PSUM") as psm_pool,
            ):
                def transpose3(dst, src, glo, ghi):
                    pst = pst_pool.tile([128, 384], BF16, tag="tp")
                    for g in range(glo, ghi):
                        nc.tensor.transpose(
                            pst[:, 128 * (g - glo):128 * (g - glo + 1)],
                            src[:, 128 * g:128 * (g + 1)], identity[:])
                    nw = 128 * (ghi - glo)
                    nc.scalar.copy(dst[:, 128 * glo:128 * ghi], pst[:, 0:nw])

                nc.vector.tensor_copy(
                    Wb4[:, :, :, 0:K], _bc(Wf4[:, :, 0:K], 2, 2))
                for pz in range(7):
                    transpose3(ZTi[:], Zslab[:], 3 * pz, 3 * pz + 3)
                transpose3(WTi[:], Wslab_b[:], 0, NFB)

                # ---------------- main loop -----------------------------
                colacc = consts.tile([128, NT], F32)
                nc.vector.memset(colacc[:], 0.0)
                tens10 = consts.tile([128, CHA], BF16)
                nc.vector.memset(tens10[:], CLAMP)

                idx = 0
                for c in range(NCH):
                    u = c // 2
                    alo = CHA * (c % 2)
                    mv_ap = ZTi[64 * u:64 * u + K, alo:alo + CHA]
                    for fb in range(NFB):
                        st_ap = WTi[64 * u:64 * u + K,
                                    128 * fb:128 * fb + 128]
                        ps = psm_pool.tile([128, CHA], F32, tag="main")
                        for mlo in range(0, CHA, 512):
                            mhi = min(mlo + 512, CHA)
                            nc.tensor.matmul(
                                ps[:, mlo:mhi], st_ap, mv_ap[:, mlo:mhi])
                        s = sbuf_s.tile([128, CHA], BF16)
                        nc.scalar.activation(
                            out=s[:], in_=ps[:], func=ACTF.Sqrt)
                        d = sbuf_s.tile([128, CHA], BF16, tag="dmin")
                        nc.vector.tensor_scalar(
                            out=d[:], in0=s[:], scalar1=CLAMP,
                            scalar2=None, op0=OP.min, op1=OP.add,
                            accum_out=colacc[:, idx:idx + 1])
                        idx += 1

                # ---------------- epilogue ------------------------------
                ScMc = consts.tile([128, 2], F32)
                nc.vector.reduce_sum(
                    out=ScMc[:, 0:1], in_=colacc[:], axis=AX.X)
                nc.vector.reduce_sum(out=ScMc[:, 1:2], in_=amb, axis=AX.X)
                ones_f = consts.tile([128, 1], F32)
                nc.vector.memset(ones_f[:], 1.0)
                psfin = psm_pool.tile([128, CHA], F32, tag="main")
                nc.tensor.matmul(psfin[0:2, 0:1], ScMc[:], ones_f[:])
                res = consts.tile([2, 1], F32)
                nc.vector.tensor_copy(res[:], psfin[0:2, 0:1])
                nc.sync.dma_start(out=out_d[:], in_=res[:])

    nc.compile()
    return nc


def prep_in_maps(inputs):
    """Full (unsharded) numpy inputs -> per-core input dicts.

    fr: per-core frame slice, [128, 72] f32, col = comp*3 + fb where the
        local frame index is 128*fb + p.
    at: atoms replicated, [128, 422] bf16, col = comp*42 + t (t = a//128,
        p = a%128), then mask [42], then a 128x128 identity.
    """
    import ml_dtypes
    f32 = np.float32
    bf16 = ml_dtypes.bfloat16

    def fr_c(x, comps, c):
        a = np.asarray(x, f32).reshape(NF, comps)[NFC * c:NFC * (c + 1)]
        return np.ascontiguousarray(
            a.reshape(NFB, 128, comps).transpose(1, 2, 0)).reshape(128, -1)

    def at_full(x, comps):
        a = np.asarray(x, f32).reshape(NA, comps)
        return np.ascontiguousarray(
            a.reshape(NAB, 128, comps).transpose(1, 2, 0)).reshape(128, -1)

    pp = at_full(inputs["predicted_atom_positions"], 3)
    tp = at_full(inputs["true_atom_positions"], 3)
    am = np.ascontiguousarray(
        np.asarray(inputs["atom_mask"], f32).reshape(NAB, 128).T)
    at = np.concatenate(
        [pp, tp, am, np.eye(128, dtype=f32)], axis=1).astype(bf16)

    in_maps = []
    for c in range(NCORES):
        fr = np.ascontiguousarray(np.concatenate([
            fr_c(inputs["predicted_frames_R"], 9, c),
            fr_c(inputs["true_frames_R"], 9, c),
            fr_c(inputs["predicted_frames_t"], 3, c),
            fr_c(inputs["true_frames_t"], 3, c),
        ], axis=1))
        in_maps.append({"fr": fr, "at": at})
    return in_maps


_NC_CACHE = None


def _get_nc():
    global _NC_CACHE
    if _NC_CACHE is None:
        _NC_CACHE = build_nc()
    return _NC_CACHE


def kernel(**inputs):
    nc = _get_nc()
    in_maps = prep_in_maps(inputs)
    r = run_bass_kernel_spmd(nc, in_maps, core_ids=list(range(NCORES)))
    S = np.float64(0.0)
    M = np.float64(0.0)
    for i in range(NCORES):
        S += np.float64(r.results[i]["out"][0, 0])
        M = np.float64(r.results[i]["out"][1, 0])
    total = S * CNORM / (EPS + M)
    return np.array([total], dtype=np.float32)
